# revision 5
# baseline (speedup 1.0000x reference)
"""AttnBlock (GroupNorm + single-head self-attention + residual) on 8 TRN2 cores.

Sharding: core = 2*b + half. Each core handles one batch element (b = core//2)
and one half of the query rows (half = core%2). The half is implemented by
swapping the token halves of x[b] host-side, so every core runs the identical
SPMD program computing outputs for local tokens [0, 2048).

Per-core device program (C=256 channels, N=4096 tokens, NH=2048 query rows):
  - GroupNorm(32 groups) via bn_stats + small PE matmuls for the cross-
    partition (8-channel) group reduction.
  - k = wk@h + bk (full), q = wq@h + bq (half), vT[m, c] = h[:,m-tile]^T @ wvT
    (producing V transposed directly, with an appended ones-column so the
    PV matmul also produces the softmax denominator).
  - S^T[m, n] = k^T q with m on partitions; exp((1/16) S^T) on ACT engine.
  - o^T[n, 0:256] (+ denom in col 256) = P^T-tiles^T @ vT-tiles, accumulated
    over 32 m-tiles in PSUM; divide by denom; PE-transpose to o[c, n].
  - out = x + wo@o + bo, DMA'd out as [256, 2048].
"""

import numpy as np

import concourse.bass as bass
import concourse.tile as tile
from concourse import bacc, mybir
from concourse.bass import ts, ds
from concourse.bass_utils import run_bass_kernel_spmd

B, C, W = 4, 256, 64
N = W * W            # 4096 tokens
NH = N // 2          # 2048 query rows per core
GROUPS = 32
GSIZE = C // GROUPS  # 8 channels per group
EPS = 1e-6
P = 128
CT = C // P          # 2 channel tiles
MT = N // P          # 32 key (m) tiles
NCH = 512            # n-chunk width for S^T / projections
SCALE = 1.0 / 16.0   # 1/sqrt(C)

F32 = mybir.dt.float32
# dtype for the two big attention matmuls (S^T and PV); fp32 = 2 cyc/row,
# bf16 = 1 cyc/row on the PE.
DT_BIG = mybir.dt.float32

AF = mybir.ActivationFunctionType
ALU = mybir.AluOpType

_CACHE = {}


def _build_program():
    nc = bacc.Bacc("TRN2", target_bir_lowering=False, debug=False, num_devices=8)

    xb = nc.dram_tensor("xb", [C, N], F32, kind="ExternalInput").ap()
    wqT = nc.dram_tensor("wqT", [C, C], F32, kind="ExternalInput").ap()
    wkT = nc.dram_tensor("wkT", [C, C], F32, kind="ExternalInput").ap()
    wvTa = nc.dram_tensor("wvTa", [C, C + 1], F32, kind="ExternalInput").ap()
    woT = nc.dram_tensor("woT", [C, C], F32, kind="ExternalInput").ap()
    bq1 = nc.dram_tensor("bq1", [C, 1], F32, kind="ExternalInput").ap()
    bk1 = nc.dram_tensor("bk1", [C, 1], F32, kind="ExternalInput").ap()
    bo1 = nc.dram_tensor("bo1", [C, 1], F32, kind="ExternalInput").ap()
    bvb = nc.dram_tensor("bvb", [P, C + 1], F32, kind="ExternalInput").ap()
    gam1 = nc.dram_tensor("gam1", [C, 1], F32, kind="ExternalInput").ap()
    bet1 = nc.dram_tensor("bet1", [C, 1], F32, kind="ExternalInput").ap()
    mfwd = nc.dram_tensor("mfwd", [P, GROUPS // CT], F32, kind="ExternalInput").ap()
    mbwd = nc.dram_tensor("mbwd", [GROUPS // CT, P], F32, kind="ExternalInput").ap()
    ident = nc.dram_tensor("ident", [P, P], F32, kind="ExternalInput").ap()
    out = nc.dram_tensor("out", [C, NH], F32, kind="ExternalOutput").ap()

    GT = GROUPS // CT  # 16 groups per channel tile

    with tile.TileContext(nc) as tc:
        with (
            tc.tile_pool(name="consts", bufs=1) as consts,
            tc.tile_pool(name="persist", bufs=1) as persist,
            tc.tile_pool(name="vt_pool", bufs=MT) as vt_pool,
        ):
            # ---- constants -------------------------------------------------
            wq_sb = consts.tile([P, CT, C], F32)
            wk_sb = consts.tile([P, CT, C], F32)
            wv_sb = consts.tile([P, CT, C + 1], F32)
            wo_sb = consts.tile([P, CT, C], F32)
            for ct in range(CT):
                nc.sync.dma_start(out=wq_sb[:, ct, :], in_=wqT[ts(ct, P), :])
                nc.sync.dma_start(out=wk_sb[:, ct, :], in_=wkT[ts(ct, P), :])
                nc.sync.dma_start(out=wv_sb[:, ct, :], in_=wvTa[ts(ct, P), :])
                nc.sync.dma_start(out=wo_sb[:, ct, :], in_=woT[ts(ct, P), :])
            bq_sb = consts.tile([P, CT], F32)
            bk_sb = consts.tile([P, CT], F32)
            bo_sb = consts.tile([P, CT], F32)
            gam_sb = consts.tile([P, CT], F32)
            bet_sb = consts.tile([P, CT], F32)
            for ct in range(CT):
                nc.sync.dma_start(out=bq_sb[:, ct : ct + 1], in_=bq1[ts(ct, P), :])
                nc.sync.dma_start(out=bk_sb[:, ct : ct + 1], in_=bk1[ts(ct, P), :])
                nc.sync.dma_start(out=bo_sb[:, ct : ct + 1], in_=bo1[ts(ct, P), :])
                nc.sync.dma_start(out=gam_sb[:, ct : ct + 1], in_=gam1[ts(ct, P), :])
                nc.sync.dma_start(out=bet_sb[:, ct : ct + 1], in_=bet1[ts(ct, P), :])
            bvb_sb = consts.tile([P, C + 1], F32)
            nc.sync.dma_start(out=bvb_sb, in_=bvb)
            mfwd_sb = consts.tile([P, GT], F32)
            nc.sync.dma_start(out=mfwd_sb, in_=mfwd)
            mbwd_sb = consts.tile([GT, P], F32)
            nc.sync.dma_start(out=mbwd_sb, in_=mbwd)
            ident_sb = consts.tile([P, P], F32)
            nc.sync.dma_start(out=ident_sb, in_=ident)
            eps_sb = consts.tile([P, 1], F32)
            nc.vector.memset(eps_sb, EPS)

            # ---- persistent activations -----------------------------------
            q_sb = [persist.tile([P, NH], DT_BIG, tag=f"q{ct}", name=f"q{ct}") for ct in range(CT)]
            k_sb = [persist.tile([P, N], DT_BIG, tag=f"k{ct}", name=f"k{ct}") for ct in range(CT)]
            oT_sb = [persist.tile([P, NH], F32, tag=f"oT{ct}", name=f"oT{ct}") for ct in range(CT)]
            vt_tiles = [vt_pool.tile([P, C + 1], DT_BIG, tag="vt", name=f"vt{mt}") for mt in range(MT)]

            # ---- prologue: load x, GroupNorm, q/k/vT projections ----------
            with (
                tc.tile_pool(name="x_pool", bufs=1) as x_pool,
                tc.tile_pool(name="h_pool", bufs=1) as h_pool,
                tc.tile_pool(name="gn_pool", bufs=2) as gn_pool,
                tc.tile_pool(name="gn_psum", bufs=1, space="PSUM") as gn_psum,
                tc.tile_pool(name="mm_psum", bufs=3, space="PSUM") as mm_psum,
            ):
                x_sb = [x_pool.tile([P, N], F32, tag=f"x{ct}", name=f"x{ct}") for ct in range(CT)]
                h_sb = [h_pool.tile([P, N], F32, tag=f"h{ct}", name=f"h{ct}") for ct in range(CT)]
                for ct in range(CT):
                    nc.sync.dma_start(out=x_sb[ct], in_=xb[ts(ct, P), :])

                for ct in range(CT):
                    xr = x_sb[ct].rearrange("p (s f) -> p s f", f=512)
                    st6 = gn_pool.tile([P, N // 512, 6], F32, tag="st6")
                    for s in range(N // 512):
                        nc.vector.bn_stats(out=st6[:, s, :], in_=xr[:, s, :])
                    mv = gn_pool.tile([P, 2], F32, tag="mv")
                    nc.vector.bn_aggr(out=mv, in_=st6)
                    # st2 = (mean_c, E[x^2]_c)
                    st2 = gn_pool.tile([P, 2], F32, tag="st2")
                    nc.vector.tensor_copy(out=st2[:, 0:1], in_=mv[:, 0:1])
                    msq = gn_pool.tile([P, 1], F32, tag="msq")
                    nc.vector.tensor_mul(out=msq, in0=mv[:, 0:1], in1=mv[:, 0:1])
                    nc.vector.tensor_add(out=st2[:, 1:2], in0=mv[:, 1:2], in1=msq)
                    # per-group (mu, E[x^2]) via 1/8-weighted column sums
                    psum_g = gn_psum.tile([GT, 2], F32, tag="pg")
                    nc.tensor.matmul(psum_g, lhsT=mfwd_sb, rhs=st2, start=True, stop=True)
                    gs = gn_pool.tile([GT, 2], F32, tag="gs")
                    nc.vector.tensor_copy(out=gs[:, 0:1], in_=psum_g[:, 0:1])
                    gv = gn_pool.tile([GT, 1], F32, tag="gv")
                    nc.vector.tensor_mul(out=gv, in0=gs[:, 0:1], in1=gs[:, 0:1])
                    nc.vector.tensor_sub(out=gv, in0=psum_g[:, 1:2], in1=gv)
                    nc.scalar.activation(
                        out=gv, in_=gv, func=AF.Sqrt, bias=eps_sb[:GT, :], scale=1.0
                    )
                    nc.vector.reciprocal(out=gs[:, 1:2], in_=gv)
                    # broadcast group stats back to channels
                    psum_bc = gn_psum.tile([P, 2], F32, tag="pbc")
                    nc.tensor.matmul(psum_bc, lhsT=mbwd_sb, rhs=gs, start=True, stop=True)
                    amul = gn_pool.tile([P, 1], F32, tag="amul")
                    badd = gn_pool.tile([P, 1], F32, tag="badd")
                    nc.vector.tensor_mul(out=amul, in0=psum_bc[:, 1:2], in1=gam_sb[:, ct : ct + 1])
                    nc.vector.tensor_mul(out=badd, in0=psum_bc[:, 0:1], in1=amul)
                    nc.vector.tensor_sub(out=badd, in0=bet_sb[:, ct : ct + 1], in1=badd)
                    nc.vector.tensor_scalar(
                        out=h_sb[ct],
                        in0=x_sb[ct],
                        scalar1=amul,
                        scalar2=badd,
                        op0=ALU.mult,
                        op1=ALU.add,
                    )

                # k (full N) and q (first NH tokens)
                for mo in range(CT):
                    for ch in range(N // NCH):
                        psk = mm_psum.tile([P, NCH], F32, tag="psk")
                        for ct in range(CT):
                            nc.tensor.matmul(
                                psk,
                                lhsT=wk_sb[:, ct, ts(mo, P)],
                                rhs=h_sb[ct][:, ts(ch, NCH)],
                                start=(ct == 0),
                                stop=(ct == CT - 1),
                            )
                        nc.scalar.activation(
                            out=k_sb[mo][:, ts(ch, NCH)],
                            in_=psk,
                            func=AF.Identity,
                            bias=bk_sb[:, mo : mo + 1],
                            scale=1.0,
                        )
                for mo in range(CT):
                    for ch in range(NH // NCH):
                        psq = mm_psum.tile([P, NCH], F32, tag="psk")
                        for ct in range(CT):
                            nc.tensor.matmul(
                                psq,
                                lhsT=wq_sb[:, ct, ts(mo, P)],
                                rhs=h_sb[ct][:, ts(ch, NCH)],
                                start=(ct == 0),
                                stop=(ct == CT - 1),
                            )
                        nc.scalar.activation(
                            out=q_sb[mo][:, ts(ch, NCH)],
                            in_=psq,
                            func=AF.Identity,
                            bias=bq_sb[:, mo : mo + 1],
                            scale=1.0,
                        )
                # vT tiles: vt[m, c] = h[:, m-tile]^T @ wvTa  (+ bias bcast)
                for mt in range(MT):
                    psv = mm_psum.tile([P, C + 1], F32, tag="psk", name="psv")
                    for ct in range(CT):
                        nc.tensor.matmul(
                            psv,
                            lhsT=h_sb[ct][:, ts(mt, P)],
                            rhs=wv_sb[:, ct, :],
                            start=(ct == 0),
                            stop=(ct == CT - 1),
                        )
                    nc.vector.tensor_add(out=vt_tiles[mt], in0=psv, in1=bvb_sb)

            # ---- main attention loop --------------------------------------
            with (
                tc.tile_pool(name="p_pool", bufs=MT) as p_pool,
                tc.tile_pool(name="s_psum", bufs=3, space="PSUM") as s_psum,
                tc.tile_pool(name="o_psum", bufs=2, space="PSUM") as o_psum,
                tc.tile_pool(name="t_psum", bufs=2, space="PSUM") as t_psum,
                tc.tile_pool(name="o_pool", bufs=3) as o_pool,
                tc.tile_pool(name="r_pool", bufs=4) as r_pool,
            ):
                for ch in range(NH // NCH):
                    pts = []
                    for mt in range(MT):
                        pss = s_psum.tile([P, NCH], F32, tag="pss")
                        for ct in range(CT):
                            nc.tensor.matmul(
                                pss,
                                lhsT=k_sb[ct][:, ts(mt, P)],
                                rhs=q_sb[ct][:, ts(ch, NCH)],
                                start=(ct == 0),
                                stop=(ct == CT - 1),
                            )
                        pt = p_pool.tile([P, NCH], DT_BIG, tag="pt", name=f"pt{mt}")
                        nc.scalar.activation(out=pt, in_=pss, func=AF.Exp, scale=SCALE)
                        pts.append(pt)
                    for nt in range(NCH // P):
                        pso = o_psum.tile([P, C + 1], F32, tag="pso")
                        for mt in range(MT):
                            nc.tensor.matmul(
                                pso,
                                lhsT=pts[mt][:, ts(nt, P)],
                                rhs=vt_tiles[mt],
                                start=(mt == 0),
                                stop=(mt == MT - 1),
                            )
                        rec = r_pool.tile([P, 1], F32, tag="rec")
                        nc.vector.reciprocal(out=rec, in_=pso[:, C : C + 1])
                        osb = o_pool.tile([P, C], F32, tag="osb")
                        nc.vector.tensor_scalar_mul(out=osb, in0=pso[:, 0:C], scalar1=rec)
                        for cc in range(CT):
                            pst = t_psum.tile([P, P], F32, tag="pst")
                            nc.tensor.transpose(pst, osb[:, ts(cc, P)], ident_sb)
                            nc.vector.tensor_copy(
                                out=oT_sb[cc][:, ds(ch * NCH + nt * P, P)], in_=pst
                            )

            # ---- epilogue: out = x + wo @ o + bo --------------------------
            with (
                tc.tile_pool(name="xh_pool", bufs=1) as xh_pool,
                tc.tile_pool(name="f_psum", bufs=3, space="PSUM") as f_psum,
                tc.tile_pool(name="out_pool", bufs=3) as out_pool,
            ):
                xh_sb = [xh_pool.tile([P, NH], F32, tag=f"xh{ct}", name=f"xh{ct}") for ct in range(CT)]
                for ct in range(CT):
                    nc.sync.dma_start(out=xh_sb[ct], in_=xb[ts(ct, P), 0:NH])
                for mo in range(CT):
                    for ch in range(NH // NCH):
                        psf = f_psum.tile([P, NCH], F32, tag="psf")
                        for ct in range(CT):
                            nc.tensor.matmul(
                                psf,
                                lhsT=wo_sb[:, ct, ts(mo, P)],
                                rhs=oT_sb[ct][:, ts(ch, NCH)],
                                start=(ct == 0),
                                stop=(ct == CT - 1),
                            )
                        fs = out_pool.tile([P, NCH], F32, tag="fs")
                        nc.scalar.activation(
                            out=fs,
                            in_=psf,
                            func=AF.Identity,
                            bias=bo_sb[:, mo : mo + 1],
                            scale=1.0,
                        )
                        nc.vector.tensor_add(out=fs, in0=fs, in1=xh_sb[mo][:, ts(ch, NCH)])
                        nc.sync.dma_start(out=out[ts(mo, P), ts(ch, NCH)], in_=fs)

    nc.compile()
    return nc


def get_program():
    if "nc" not in _CACHE:
        _CACHE["nc"] = _build_program()
    return _CACHE["nc"]


def _make_in_maps(x, gn_gamma, gn_beta, wq, bq, wk, bk, wv, bv, wo, bo):
    f = lambda a: np.ascontiguousarray(np.asarray(a, dtype=np.float32))
    x = f(x).reshape(B, C, N)
    shared = {
        "wqT": f(wq).T.copy(),
        "wkT": f(wk).T.copy(),
        "wvTa": np.concatenate([f(wv).T, np.zeros((C, 1), np.float32)], axis=1),
        "woT": f(wo).T.copy(),
        "bq1": f(bq).reshape(C, 1),
        "bk1": f(bk).reshape(C, 1),
        "bo1": f(bo).reshape(C, 1),
        "bvb": np.concatenate(
            [np.broadcast_to(f(bv), (P, C)), np.ones((P, 1), np.float32)], axis=1
        ).copy(),
        "gam1": f(gn_gamma).reshape(C, 1),
        "bet1": f(gn_beta).reshape(C, 1),
        # 1/GSIZE for channels in the group: group stats = mean of 8 channel stats
        "mfwd": (
            (np.arange(P)[:, None] // GSIZE == np.arange(GROUPS // CT)[None, :]).astype(
                np.float32
            )
            / GSIZE
        ),
        "mbwd": (np.arange(GROUPS // CT)[:, None] == np.arange(P)[None, :] // GSIZE)
        .astype(np.float32),
        "ident": np.eye(P, dtype=np.float32),
    }
    in_maps = []
    for core in range(8):
        b, half = core // 2, core % 2
        xbv = x[b]
        if half == 1:
            xbv = np.concatenate([xbv[:, NH:], xbv[:, :NH]], axis=1)
        in_maps.append({"xb": np.ascontiguousarray(xbv), **shared})
    return in_maps


def kernel(**inputs):
    nc = get_program()
    in_maps = _make_in_maps(**inputs)
    res = run_bass_kernel_spmd(nc, in_maps, list(range(8)))
    out = np.empty((B, C, N), dtype=np.float32)
    for core in range(8):
        b, half = core // 2, core % 2
        out[b, :, half * NH : (half + 1) * NH] = res.results[core]["out"]
    return out.reshape(B, C, W, W)


# revision 11
# speedup vs baseline: 71.0290x; 71.0290x over previous
"""AttnBlock (GroupNorm + single-head self-attention + residual) on 8 TRN2 cores.

Sharding: core = 2*b + half. Each core handles one batch element (b = core//2)
and one half of the query rows (half = core%2). The half is implemented by
swapping the token halves of x[b] host-side, so every core runs the identical
SPMD program computing outputs for local tokens [0, 2048).

Per-core device program (C=256 channels, N=4096 tokens, NH=2048 query rows):
  - GroupNorm(32 groups) via bn_stats + small PE matmuls for the cross-
    partition (8-channel) group reduction.
  - k = wk@h + bk (full), q = wq@h + bq (half), vT[m, c] = h[:,m-tile]^T @ wvT
    (producing V transposed directly, with an appended ones-column so the
    PV matmul also produces the softmax denominator).
  - S^T[m, n] = k^T q with m on partitions; exp((1/16) S^T) on ACT engine.
  - o^T[n, 0:256] (+ denom in col 256) = P^T-tiles^T @ vT-tiles, accumulated
    over 32 m-tiles in PSUM; divide by denom; PE-transpose to o[c, n];
    out = x + wo@o + bo computed per 512-column chunk inside the main loop.

All large matmuls run in bf16 (1 PE cycle/row vs 4 for fp32); accumulation is
fp32 in PSUM, GroupNorm statistics and the residual path stay fp32. The final
output error is dominated by the fp32 residual since |wo| ~ 1e-5.
"""

import ml_dtypes
import numpy as np

import concourse.bass as bass
import concourse.tile as tile
from concourse import bacc, mybir
from concourse.bass import ts, ds
from concourse.bass_utils import run_bass_kernel_spmd

B, C, W = 4, 256, 64
N = W * W            # 4096 tokens
NH = N // 2          # 2048 query rows per core
GROUPS = 32
GSIZE = C // GROUPS  # 8 channels per group
EPS = 1e-6
P = 128
CT = C // P          # 2 channel tiles
MT = N // P          # 32 key (m) tiles
NCH = 512            # n-chunk width for S^T / projections
SCALE = 1.0 / 16.0   # 1/sqrt(C)

F32 = mybir.dt.float32
BF = mybir.dt.bfloat16

AF = mybir.ActivationFunctionType
ALU = mybir.AluOpType

_CACHE = {}


def _build_program():
    nc = bacc.Bacc("TRN2", target_bir_lowering=False, debug=False, num_devices=8)

    xb = nc.dram_tensor("xb", [C, N], F32, kind="ExternalInput").ap()
    wqT = nc.dram_tensor("wqT", [C, C], BF, kind="ExternalInput").ap()
    wkT = nc.dram_tensor("wkT", [C, C], BF, kind="ExternalInput").ap()
    wvTa = nc.dram_tensor("wvTa", [C, C + 1], BF, kind="ExternalInput").ap()
    woT = nc.dram_tensor("woT", [C, C], BF, kind="ExternalInput").ap()
    # all small fp32 constants packed in one tensor: one DMA instead of ~15.
    # layout: [0:10] per-ct (bq, bk, bo, gamma, beta), [10:26] mfwd,
    # [26:154] mbwd (partitions 0:16 valid), [154:411] bvb
    CPK = 10 + 16 + P + (C + 1)
    cpack = nc.dram_tensor("cpack", [P, CPK], F32, kind="ExternalInput").ap()
    ident = nc.dram_tensor("ident", [P, P], BF, kind="ExternalInput").ap()
    out = nc.dram_tensor("out", [C, NH], F32, kind="ExternalOutput").ap()

    GT = GROUPS // CT  # 16 groups per channel tile

    with tile.TileContext(nc) as tc:
        with (
            tc.tile_pool(name="persist", bufs=1) as persist,
            tc.tile_pool(name="consts", bufs=1) as consts,
            tc.tile_pool(name="vt_pool", bufs=MT) as vt_pool,
        ):
            # ---- x load first: GroupNorm is the head of the dependency chain
            x_sb = [persist.tile([P, N], F32, tag=f"x{ct}", name=f"x{ct}") for ct in range(CT)]
            for hh in range(2):
                for ct in range(CT):
                    eng = nc.sync if ct == 0 else nc.gpsimd
                    eng.dma_start(
                        out=x_sb[ct][:, ts(hh, N // 2)],
                        in_=xb[ts(ct, P), ts(hh, N // 2)],
                    )
            cpack_sb = consts.tile([P, CPK], F32)
            nc.sync.dma_start(out=cpack_sb, in_=cpack)

            # ---- constants (sync queue, behind x) -------------------------
            wq_sb = consts.tile([P, CT, C], BF)
            wk_sb = consts.tile([P, CT, C], BF)
            wv_sb = consts.tile([P, CT, C + 1], BF)
            wo_sb = consts.tile([P, CT, C], BF)
            for ct in range(CT):
                nc.sync.dma_start(out=wk_sb[:, ct, :], in_=wkT[ts(ct, P), :])
                nc.sync.dma_start(out=wq_sb[:, ct, :], in_=wqT[ts(ct, P), :])
                nc.sync.dma_start(out=wv_sb[:, ct, :], in_=wvTa[ts(ct, P), :])
                nc.sync.dma_start(out=wo_sb[:, ct, :], in_=woT[ts(ct, P), :])
            ident_sb = consts.tile([P, P], BF)
            nc.sync.dma_start(out=ident_sb, in_=ident)
            eps_sb = consts.tile([P, 1], F32)
            nc.vector.memset(eps_sb, EPS)
            # views into the packed constants
            bq_sb = cpack_sb[:, 0:CT]
            bk_sb = cpack_sb[:, CT : 2 * CT]
            bo_sb = cpack_sb[:, 2 * CT : 3 * CT]
            gam_sb = cpack_sb[:, 3 * CT : 4 * CT]
            bet_sb = cpack_sb[:, 4 * CT : 5 * CT]
            mfwd_sb = cpack_sb[:, 10 : 10 + GT]
            mbwd_sb = cpack_sb[0:GT, 26 : 26 + P]
            bvb_sb = cpack_sb[:, 154 : 154 + C + 1]

            # ---- persistent activations -----------------------------------
            q_sb = [persist.tile([P, NH], BF, tag=f"q{ct}", name=f"q{ct}") for ct in range(CT)]
            k_sb = [persist.tile([P, N], BF, tag=f"k{ct}", name=f"k{ct}") for ct in range(CT)]
            h_sb = [persist.tile([P, N], BF, tag=f"h{ct}", name=f"h{ct}") for ct in range(CT)]
            oT_sb = [persist.tile([P, NH], BF, tag=f"oT{ct}", name=f"oT{ct}") for ct in range(CT)]
            vt_tiles = [vt_pool.tile([P, C + 1], BF, tag="vt", name=f"vt{mt}") for mt in range(MT)]
            # residual reload via SWDGE (gpsimd) so it never blocks the sync
            # queue; only consumed at the end of each chunk's projection.
            xh_sb = [persist.tile([P, NH], F32, tag=f"xh{ct}", name=f"xh{ct}") for ct in range(CT)]
            for ct in range(CT):
                nc.gpsimd.dma_start(out=xh_sb[ct], in_=xb[ts(ct, P), 0:NH])

            # ---- GroupNorm -------------------------------------------------
            with (
                tc.tile_pool(name="gn_pool", bufs=2) as gn_pool,
                tc.tile_pool(name="gn_psum", bufs=1, space="PSUM") as gn_psum,
                tc.tile_pool(name="mm_psum", bufs=3, space="PSUM") as mm_psum,
            ):
                for ct in range(CT):
                    xr = x_sb[ct].rearrange("p (s f) -> p s f", f=512)
                    st6 = gn_pool.tile([P, N // 512, 6], F32, tag="st6")
                    for s in range(N // 512):
                        nc.vector.bn_stats(out=st6[:, s, :], in_=xr[:, s, :])
                    mv = gn_pool.tile([P, 2], F32, tag="mv")
                    nc.vector.bn_aggr(out=mv, in_=st6)
                    # st2 = (mean_c, E[x^2]_c)
                    st2 = gn_pool.tile([P, 2], F32, tag="st2")
                    nc.vector.tensor_copy(out=st2[:, 0:1], in_=mv[:, 0:1])
                    msq = gn_pool.tile([P, 1], F32, tag="msq")
                    nc.vector.tensor_mul(out=msq, in0=mv[:, 0:1], in1=mv[:, 0:1])
                    nc.vector.tensor_add(out=st2[:, 1:2], in0=mv[:, 1:2], in1=msq)
                    # per-group (mu, E[x^2]) via 1/8-weighted column sums
                    psum_g = gn_psum.tile([GT, 2], F32, tag="pg")
                    nc.tensor.matmul(psum_g, lhsT=mfwd_sb, rhs=st2, start=True, stop=True)
                    gs = gn_pool.tile([GT, 2], F32, tag="gs")
                    nc.vector.tensor_copy(out=gs[:, 0:1], in_=psum_g[:, 0:1])
                    gv = gn_pool.tile([GT, 1], F32, tag="gv")
                    nc.vector.tensor_mul(out=gv, in0=gs[:, 0:1], in1=gs[:, 0:1])
                    nc.vector.tensor_sub(out=gv, in0=psum_g[:, 1:2], in1=gv)
                    nc.scalar.activation(
                        out=gv, in_=gv, func=AF.Sqrt, bias=eps_sb[:GT, :], scale=1.0
                    )
                    nc.vector.reciprocal(out=gs[:, 1:2], in_=gv)
                    # broadcast group stats back to channels
                    psum_bc = gn_psum.tile([P, 2], F32, tag="pbc")
                    nc.tensor.matmul(psum_bc, lhsT=mbwd_sb, rhs=gs, start=True, stop=True)
                    amul = gn_pool.tile([P, 1], F32, tag="amul")
                    badd = gn_pool.tile([P, 1], F32, tag="badd")
                    nc.vector.tensor_mul(out=amul, in0=psum_bc[:, 1:2], in1=gam_sb[:, ct : ct + 1])
                    nc.vector.tensor_mul(out=badd, in0=psum_bc[:, 0:1], in1=amul)
                    nc.vector.tensor_sub(out=badd, in0=bet_sb[:, ct : ct + 1], in1=badd)
                    # h = x*A + B, in 1024-wide pieces so QKV can start early;
                    # ct0 goes on ACT so it overlaps ct1's stats on DVE
                    for s4 in range(4):
                        if ct == 0:
                            nc.scalar.activation(
                                out=h_sb[ct][:, ts(s4, N // 4)],
                                in_=x_sb[ct][:, ts(s4, N // 4)],
                                func=AF.Identity,
                                bias=badd,
                                scale=amul,
                            )
                        else:
                            nc.vector.tensor_scalar(
                                out=h_sb[ct][:, ts(s4, N // 4)],
                                in0=x_sb[ct][:, ts(s4, N // 4)],
                                scalar1=amul,
                                scalar2=badd,
                                op0=ALU.mult,
                                op1=ALU.add,
                            )

                # ---- q/k/vT projections, interleaved so the ACT (k/q copies)
                # and DVE (vT bias-adds) consumers stay balanced ------------
                for ch in range(N // NCH):
                    psk = mm_psum.tile([P, NCH], F32, tag="psk", name="psk")
                    for mo in range(CT):
                        if mo > 0:
                            psk = mm_psum.tile([P, NCH], F32, tag="psk", name="psk2")
                        for ct in range(CT):
                            nc.tensor.matmul(
                                psk,
                                lhsT=wk_sb[:, ct, ts(mo, P)],
                                rhs=h_sb[ct][:, ts(ch, NCH)],
                                start=(ct == 0),
                                stop=(ct == CT - 1),
                            )
                        nc.scalar.activation(
                            out=k_sb[mo][:, ts(ch, NCH)],
                            in_=psk,
                            func=AF.Identity,
                            bias=bk_sb[:, mo : mo + 1],
                            scale=1.0,
                        )
                    if ch < NH // NCH:
                        for mo in range(CT):
                            psq = mm_psum.tile([P, NCH], F32, tag="psk", name="psq")
                            for ct in range(CT):
                                nc.tensor.matmul(
                                    psq,
                                    lhsT=wq_sb[:, ct, ts(mo, P)],
                                    rhs=h_sb[ct][:, ts(ch, NCH)],
                                    start=(ct == 0),
                                    stop=(ct == CT - 1),
                                )
                            nc.scalar.activation(
                                out=q_sb[mo][:, ts(ch, NCH)],
                                in_=psq,
                                func=AF.Identity,
                                bias=bq_sb[:, mo : mo + 1],
                                scale=1.0,
                            )
                    for mt in range(4 * ch, 4 * ch + 4):
                        psv = mm_psum.tile([P, C + 1], F32, tag="psk", name="psv")
                        for ct in range(CT):
                            nc.tensor.matmul(
                                psv,
                                lhsT=h_sb[ct][:, ts(mt, P)],
                                rhs=wv_sb[:, ct, :],
                                start=(ct == 0),
                                stop=(ct == CT - 1),
                            )
                        nc.vector.tensor_add(out=vt_tiles[mt], in0=psv, in1=bvb_sb)

            # ---- main attention loop (with fused output projection) -------
            with (
                tc.tile_pool(name="p_pool", bufs=MT) as p_pool,
                tc.tile_pool(name="s_psum", bufs=2, space="PSUM") as s_psum,
                tc.tile_pool(name="o_psum", bufs=4, space="PSUM") as o_psum,
                tc.tile_pool(name="tf_psum", bufs=2, space="PSUM") as tf_psum,
                tc.tile_pool(name="o_pool", bufs=3) as o_pool,
                tc.tile_pool(name="r_pool", bufs=4) as r_pool,
                tc.tile_pool(name="out_pool", bufs=4) as out_pool,
            ):
                LAG = 2
                NT = NCH // P  # 4 n-tiles per chunk
                NCHUNKS = NH // NCH

                for ch in range(NCHUNKS):
                    last = ch == NCHUNKS - 1
                    pts = []
                    psos = [o_psum.tile([P, C + 1], F32, tag="pso", name=f"pso{nt}") for nt in range(NT)]
                    # on the last chunk, only chains 0/1 interleave behind S;
                    # chains 2/3 run after, so the divide->transpose epilogues
                    # of earlier n-tiles overlap PE instead of trailing it.
                    inter_nts = [0, 1] if last else list(range(NT))

                    def pv(mt, nts):
                        for nt in nts:
                            nc.tensor.matmul(
                                psos[nt],
                                lhsT=pts[mt][:, ts(nt, P)],
                                rhs=vt_tiles[mt],
                                start=(mt == 0),
                                stop=(mt == MT - 1),
                            )

                    def finish_nt(nt):
                        rec = r_pool.tile([P, 1], F32, tag="rec", name=f"rec{nt}")
                        nc.vector.reciprocal(out=rec, in_=psos[nt][:, C : C + 1])
                        osb = o_pool.tile([P, C], BF, tag="osb", name=f"osb{nt}")
                        nc.vector.tensor_scalar_mul(out=osb, in0=psos[nt][:, 0:C], scalar1=rec)
                        for cc in range(CT):
                            pst = tf_psum.tile([P, P], BF, tag="psf", name=f"pst{nt}{cc}")
                            nc.tensor.transpose(pst, osb[:, ts(cc, P)], ident_sb)
                            nc.vector.tensor_copy(
                                out=oT_sb[cc][:, ds(ch * NCH + nt * P, P)], in_=pst
                            )

                    for mt in range(MT):
                        pss = s_psum.tile([P, NCH], F32, tag="pss")
                        for ct in range(CT):
                            nc.tensor.matmul(
                                pss,
                                lhsT=k_sb[ct][:, ts(mt, P)],
                                rhs=q_sb[ct][:, ts(ch, NCH)],
                                start=(ct == 0),
                                stop=(ct == CT - 1),
                            )
                        pt = p_pool.tile([P, NCH], BF, tag="pt", name=f"pt{mt}")
                        nc.scalar.activation(out=pt, in_=pss, func=AF.Exp, scale=SCALE)
                        pts.append(pt)
                        if mt >= LAG:
                            pv(mt - LAG, inter_nts)
                    for mt in range(MT - LAG, MT):
                        pv(mt, inter_nts)
                    if last:
                        for mt in range(MT):
                            pv(mt, [2])
                        finish_nt(0)
                        finish_nt(1)
                        for mt in range(MT):
                            pv(mt, [3])
                        finish_nt(2)
                        finish_nt(3)
                    else:
                        for nt in range(NT):
                            finish_nt(nt)
                    # output projection + residual for this chunk
                    for mo in range(CT):
                        psf = tf_psum.tile([P, NCH], F32, tag="psf")
                        for ct in range(CT):
                            nc.tensor.matmul(
                                psf,
                                lhsT=wo_sb[:, ct, ts(mo, P)],
                                rhs=oT_sb[ct][:, ts(ch, NCH)],
                                start=(ct == 0),
                                stop=(ct == CT - 1),
                            )
                        fs = out_pool.tile([P, NCH], F32, tag="fs")
                        nc.scalar.activation(
                            out=fs,
                            in_=psf,
                            func=AF.Identity,
                            bias=bo_sb[:, mo : mo + 1],
                            scale=1.0,
                        )
                        nc.vector.tensor_add(out=fs, in0=fs, in1=xh_sb[mo][:, ts(ch, NCH)])
                        nc.sync.dma_start(out=out[ts(mo, P), ts(ch, NCH)], in_=fs)

    nc.compile()
    return nc


def get_program():
    if "nc" not in _CACHE:
        _CACHE["nc"] = _build_program()
    return _CACHE["nc"]


def _cpack(bq, bk, bo, gam, bet, bv):
    cp = np.zeros((P, 10 + 16 + P + C + 1), np.float32)
    for j, v in enumerate([bq, bk, bo, gam, bet]):
        cp[:, 2 * j : 2 * j + 2] = v.reshape(CT, P).T
    mfwd = (
        np.arange(P)[:, None] // GSIZE == np.arange(GROUPS // CT)[None, :]
    ).astype(np.float32) / GSIZE
    mbwd = (
        np.arange(GROUPS // CT)[:, None] == np.arange(P)[None, :] // GSIZE
    ).astype(np.float32)
    cp[:, 10:26] = mfwd
    cp[: GROUPS // CT, 26 : 26 + P] = mbwd
    cp[:, 154 : 154 + C] = np.broadcast_to(bv, (P, C))
    cp[:, 154 + C] = 1.0
    return cp


def _make_in_maps(x, gn_gamma, gn_beta, wq, bq, wk, bk, wv, bv, wo, bo):
    f = lambda a: np.ascontiguousarray(np.asarray(a, dtype=np.float32))
    x = f(x).reshape(B, C, N)
    shared = {
        "wqT": f(wq).T.astype(ml_dtypes.bfloat16),
        "wkT": f(wk).T.astype(ml_dtypes.bfloat16),
        "wvTa": np.concatenate(
            [f(wv).T, np.zeros((C, 1), np.float32)], axis=1
        ).astype(ml_dtypes.bfloat16),
        "woT": f(wo).T.astype(ml_dtypes.bfloat16),
        "cpack": _cpack(f(bq), f(bk), f(bo), f(gn_gamma), f(gn_beta), f(bv)),
        "ident": np.eye(P).astype(ml_dtypes.bfloat16),
    }
    in_maps = []
    for core in range(8):
        b, half = core // 2, core % 2
        xbv = x[b]
        if half == 1:
            xbv = np.concatenate([xbv[:, NH:], xbv[:, :NH]], axis=1)
        in_maps.append({"xb": np.ascontiguousarray(xbv), **shared})
    return in_maps


def kernel(**inputs):
    nc = get_program()
    in_maps = _make_in_maps(**inputs)
    res = run_bass_kernel_spmd(nc, in_maps, list(range(8)))
    out = np.empty((B, C, N), dtype=np.float32)
    for core in range(8):
        b, half = core // 2, core % 2
        out[b, :, half * NH : (half + 1) * NH] = res.results[core]["out"]
    return out.reshape(B, C, W, W)


# revision 25
# speedup vs baseline: 10069.4524x; 141.7654x over previous
"""AttnBlock (GroupNorm + single-head self-attention + residual) on 8 TRN2 cores.

Sharding: core = 2*b + half. Each core handles one batch element (b = core//2)
and one half of the query rows (half = core%2). The half is implemented by
swapping the token halves of x[b] host-side, so every core runs the identical
SPMD program computing outputs for local tokens [0, 2048).

Per-core device program (C=256 channels, N=4096 tokens, NH=2048 query rows):
  - GroupNorm(32 groups) via bn_stats + small PE matmuls for the cross-
    partition (8-channel) group reduction.
  - k = wk@h + bk (full), q = wq@h + bq (half), vT[m, c] = h[:,m-tile]^T @ wvT
    (producing V transposed directly, with an appended ones-column so the
    PV matmul also produces the softmax denominator).
  - S^T[m, n] = k^T q with m on partitions; exp((1/16) S^T) on ACT engine.
  - o^T[n, 0:256] (+ denom in col 256) = P^T-tiles^T @ vT-tiles, accumulated
    over 32 m-tiles in PSUM; divide by denom; PE-transpose to o[c, n];
    out = x + wo@o + bo computed per 512-column chunk inside the main loop.

All large matmuls run in bf16 (1 PE cycle/row vs 4 for fp32); accumulation is
fp32 in PSUM, GroupNorm statistics and the residual path stay fp32. The final
output error is dominated by the fp32 residual since |wo| ~ 1e-5.
"""

import ml_dtypes
import numpy as np

import concourse.bass as bass
import concourse.tile as tile
from concourse import bacc, mybir
from concourse.bass import ts, ds
from concourse.bass_utils import run_bass_kernel_spmd

B, C, W = 4, 256, 64
N = W * W            # 4096 tokens
NH = N // 2          # 2048 query rows per core
GROUPS = 32
GSIZE = C // GROUPS  # 8 channels per group
EPS = 1e-6
P = 128
CT = C // P          # 2 channel tiles
MT = N // P          # 32 key (m) tiles
NCH = 512            # n-chunk width for S^T / projections
SCALE = 1.0 / 16.0   # 1/sqrt(C)

F32 = mybir.dt.float32
BF = mybir.dt.bfloat16
F8 = mybir.dt.float8e4
PMT = 16  # packed key-token tiles (256 tokens each, even/odd planes)

AF = mybir.ActivationFunctionType
ALU = mybir.AluOpType

_CACHE = {}


def _build_program():
    nc = bacc.Bacc("TRN2", target_bir_lowering=False, debug=False, num_devices=8)

    xb = nc.dram_tensor("xb", [C, NH], F32, kind="ExternalInput").ap()
    xhb = nc.dram_tensor("xhb", [C, NH], BF, kind="ExternalInput").ap()
    wqT = nc.dram_tensor("wqT", [C, C], BF, kind="ExternalInput").ap()
    wkT = nc.dram_tensor("wkT", [C, C], BF, kind="ExternalInput").ap()
    wvTa = nc.dram_tensor("wvTa", [C, C + 1], BF, kind="ExternalInput").ap()
    woT = nc.dram_tensor("woT", [C, C], BF, kind="ExternalInput").ap()
    # all small fp32 constants packed in one tensor: one DMA instead of ~15.
    # layout: [0:10] per-ct (bq, bk, bo, gamma, beta), [10:26] mfwd,
    # [26:154] mbwd (partitions 0:16 valid), [154:411] bvb
    CPK = 10 + 16 + P + (C + 1)
    cpack = nc.dram_tensor("cpack", [P, CPK], F32, kind="ExternalInput").ap()
    ident = nc.dram_tensor("ident", [P, P], BF, kind="ExternalInput").ap()
    out = nc.dram_tensor("out", [C, NH], F32, kind="ExternalOutput").ap()

    GT = GROUPS // CT  # 16 groups per channel tile

    with tile.TileContext(nc) as tc:
        with (
            tc.tile_pool(name="persist", bufs=1) as persist,
            tc.tile_pool(name="consts", bufs=1) as consts,
            tc.tile_pool(name="vt_pool", bufs=PMT) as vt_pool,
        ):
            # ---- x load first: GroupNorm is the head of the dependency chain
            x_sb = [persist.tile([P, NH], F32, tag=f"x{ct}", name=f"x{ct}") for ct in range(CT)]
            xh_sb = [persist.tile([P, NH], BF, tag=f"xh{ct}", name=f"xh{ct}") for ct in range(CT)]
            for hh in range(2):
                for ct in range(CT):
                    eng = nc.sync if ct == 0 else nc.gpsimd
                    eng.dma_start(
                        out=x_sb[ct][:, ts(hh, NH // 2)],
                        in_=xb[ts(ct, P), ts(hh, NH // 2)],
                    )
            for hh in range(2):
                for ct in range(CT):
                    eng = nc.sync if ct == 0 else nc.gpsimd
                    eng.dma_start(
                        out=xh_sb[ct][:, ts(hh, NH // 2)],
                        in_=xhb[ts(ct, P), ts(hh, NH // 2)],
                    )
            cpack_sb = consts.tile([P, CPK], F32)
            nc.sync.dma_start(out=cpack_sb, in_=cpack)

            # ---- constants (sync queue, behind x) -------------------------
            wq_sb = consts.tile([P, CT, C], BF)
            wk_sb = consts.tile([P, CT, C], BF)
            wv_sb = consts.tile([P, CT, C + 1], BF)
            wo_sb = consts.tile([P, CT, C], BF)
            for ct in range(CT):
                nc.sync.dma_start(out=wk_sb[:, ct, :], in_=wkT[ts(ct, P), :])
                nc.sync.dma_start(out=wq_sb[:, ct, :], in_=wqT[ts(ct, P), :])
                nc.sync.dma_start(out=wv_sb[:, ct, :], in_=wvTa[ts(ct, P), :])
                nc.sync.dma_start(out=wo_sb[:, ct, :], in_=woT[ts(ct, P), :])
            ident_sb = consts.tile([P, P], BF)
            nc.sync.dma_start(out=ident_sb, in_=ident)
            eps_sb = consts.tile([P, 1], F32)
            nc.vector.memset(eps_sb, EPS)
            # constant bias inside exp keeps fp8 attention weights in range
            # (max score/16 ~ 5.5 -> exp up to ~450 overflows e4m3); the e^-2
            # factor cancels exactly in the softmax ratio.
            nexp_sb = consts.tile([P, 1], F32)
            nc.vector.memset(nexp_sb, -2.0)
            # views into the packed constants
            bq_sb = cpack_sb[:, 0:CT]
            bk_sb = cpack_sb[:, CT : 2 * CT]
            bo_sb = cpack_sb[:, 2 * CT : 3 * CT]
            gam_sb = cpack_sb[:, 3 * CT : 4 * CT]
            bet_sb = cpack_sb[:, 4 * CT : 5 * CT]
            mfwd_sb = cpack_sb[:, 10 : 10 + GT]
            mbwd_sb = cpack_sb[0:GT, 26 : 26 + P]
            bvb_sb = cpack_sb[:, 154 : 154 + C + 1]

            # ---- persistent activations -----------------------------------
            q_sb = [persist.tile([P, NH], BF, tag=f"q{ct}", name=f"q{ct}") for ct in range(CT)]
            k_sb = [persist.tile([P, N], BF, tag=f"k{ct}", name=f"k{ct}") for ct in range(CT)]
            h_sb = [persist.tile([P, N], BF, tag=f"h{ct}", name=f"h{ct}") for ct in range(CT)]
            oT_sb = [persist.tile([P, NH], BF, tag=f"oT{ct}", name=f"oT{ct}") for ct in range(CT)]
            vt_tiles = [vt_pool.tile([P, 2, C + 1], F8, tag="vt", name=f"vt{j}") for j in range(PMT)]

            # ---- GroupNorm -------------------------------------------------
            with (
                tc.tile_pool(name="gn_pool", bufs=2) as gn_pool,
                tc.tile_pool(name="gn_psum", bufs=1, space="PSUM") as gn_psum,
                tc.tile_pool(name="mm_psum", bufs=4, space="PSUM") as mm_psum,
            ):
                for ct in range(CT):
                    xr = x_sb[ct].rearrange("p (s f) -> p s f", f=512)
                    xhr = xh_sb[ct].rearrange("p (s f) -> p s f", f=512)
                    st6 = gn_pool.tile([P, N // 512, 6], F32, tag="st6")
                    for s in range(NH // 512):
                        nc.vector.bn_stats(out=st6[:, s, :], in_=xr[:, s, :])
                    for s in range(NH // 512):
                        nc.vector.bn_stats(
                            out=st6[:, NH // 512 + s, :], in_=xhr[:, s, :]
                        )
                    mv = gn_pool.tile([P, 2], F32, tag="mv")
                    nc.vector.bn_aggr(out=mv, in_=st6)
                    # st2 = (mean_c, E[x^2]_c)
                    st2 = gn_pool.tile([P, 2], F32, tag="st2")
                    nc.vector.tensor_copy(out=st2[:, 0:1], in_=mv[:, 0:1])
                    msq = gn_pool.tile([P, 1], F32, tag="msq")
                    nc.vector.tensor_mul(out=msq, in0=mv[:, 0:1], in1=mv[:, 0:1])
                    nc.vector.tensor_add(out=st2[:, 1:2], in0=mv[:, 1:2], in1=msq)
                    # per-group (mu, E[x^2]) via 1/8-weighted column sums
                    psum_g = gn_psum.tile([GT, 2], F32, tag="pg")
                    nc.tensor.matmul(psum_g, lhsT=mfwd_sb, rhs=st2, start=True, stop=True)
                    gs = gn_pool.tile([GT, 2], F32, tag="gs")
                    nc.vector.tensor_copy(out=gs[:, 0:1], in_=psum_g[:, 0:1])
                    gv = gn_pool.tile([GT, 1], F32, tag="gv")
                    nc.vector.tensor_mul(out=gv, in0=gs[:, 0:1], in1=gs[:, 0:1])
                    nc.vector.tensor_sub(out=gv, in0=psum_g[:, 1:2], in1=gv)
                    nc.scalar.activation(
                        out=gv, in_=gv, func=AF.Sqrt, bias=eps_sb[:GT, :], scale=1.0
                    )
                    nc.vector.reciprocal(out=gs[:, 1:2], in_=gv)
                    # broadcast group stats back to channels
                    psum_bc = gn_psum.tile([P, 2], F32, tag="pbc")
                    nc.tensor.matmul(psum_bc, lhsT=mbwd_sb, rhs=gs, start=True, stop=True)
                    amul = gn_pool.tile([P, 1], F32, tag="amul")
                    badd = gn_pool.tile([P, 1], F32, tag="badd")
                    nc.vector.tensor_mul(out=amul, in0=psum_bc[:, 1:2], in1=gam_sb[:, ct : ct + 1])
                    nc.vector.tensor_mul(out=badd, in0=psum_bc[:, 0:1], in1=amul)
                    nc.vector.tensor_sub(out=badd, in0=bet_sb[:, ct : ct + 1], in1=badd)
                    # h = x*A + B, in 1024-wide pieces so QKV can start early;
                    # ct0 goes on ACT so it overlaps ct1's stats on DVE
                    for s4 in range(4):
                        src_t = x_sb[ct] if s4 < 2 else xh_sb[ct]
                        sl = ts(s4 % 2, NH // 2)
                        if ct == 0:
                            nc.scalar.activation(
                                out=h_sb[ct][:, ts(s4, N // 4)],
                                in_=src_t[:, sl],
                                func=AF.Identity,
                                bias=badd,
                                scale=amul,
                            )
                        else:
                            nc.vector.tensor_scalar(
                                out=h_sb[ct][:, ts(s4, N // 4)],
                                in0=src_t[:, sl],
                                scalar1=amul,
                                scalar2=badd,
                                op0=ALU.mult,
                                op1=ALU.add,
                            )

                # ---- q/k/vT projections, interleaved so the ACT (k/q copies)
                # and DVE (vT bias-adds) consumers stay balanced ------------
                for ch in range(N // NCH):
                    psk = mm_psum.tile([P, NCH], F32, tag="psk", name="psk")
                    for mo in range(CT):
                        if mo > 0:
                            psk = mm_psum.tile([P, NCH], F32, tag="psk", name="psk2")
                        for ct in range(CT):
                            nc.tensor.matmul(
                                psk,
                                lhsT=wk_sb[:, ct, ts(mo, P)],
                                rhs=h_sb[ct][:, ts(ch, NCH)],
                                start=(ct == 0),
                                stop=(ct == CT - 1),
                            )
                        nc.scalar.activation(
                            out=k_sb[mo][:, ts(ch, NCH)],
                            in_=psk,
                            func=AF.Identity,
                            bias=bk_sb[:, mo : mo + 1],
                            scale=1.0,
                        )
                    if ch < NH // NCH:
                        for mo in range(CT):
                            psq = mm_psum.tile([P, NCH], F32, tag="psk", name="psq")
                            for ct in range(CT):
                                nc.tensor.matmul(
                                    psq,
                                    lhsT=wq_sb[:, ct, ts(mo, P)],
                                    rhs=h_sb[ct][:, ts(ch, NCH)],
                                    start=(ct == 0),
                                    stop=(ct == CT - 1),
                                )
                            nc.scalar.activation(
                                out=q_sb[mo][:, ts(ch, NCH)],
                                in_=psq,
                                func=AF.Identity,
                                bias=bq_sb[:, mo : mo + 1],
                                scale=1.0,
                            )
                    for j in (2 * ch, 2 * ch + 1):
                        for parity in range(2):
                            psv = mm_psum.tile([P, C + 1], F32, tag="psk", name="psv")
                            for ct in range(CT):
                                hsl = h_sb[ct][:, ds(j * 2 * P, 2 * P)].rearrange(
                                    "p (m two) -> p two m", two=2
                                )
                                nc.tensor.matmul(
                                    psv,
                                    lhsT=hsl[:, parity, :],
                                    rhs=wv_sb[:, ct, :],
                                    start=(ct == 0),
                                    stop=(ct == CT - 1),
                                )
                            nc.vector.tensor_add(
                                out=vt_tiles[j][:, parity, :], in0=psv, in1=bvb_sb
                            )

            # ---- main attention loop (with fused output projection) -------
            with (
                tc.tile_pool(name="p_pool", bufs=PMT) as p_pool,
                tc.tile_pool(name="s_psum", bufs=2, space="PSUM") as s_psum,
                tc.tile_pool(name="o_psum", bufs=4, space="PSUM") as o_psum,
                tc.tile_pool(name="tf_psum", bufs=2, space="PSUM") as tf_psum,
                tc.tile_pool(name="o_pool", bufs=3) as o_pool,
                tc.tile_pool(name="r_pool", bufs=4) as r_pool,
                tc.tile_pool(name="out_pool", bufs=4) as out_pool,
            ):
                LAG = 2
                NT = NCH // P  # 4 n-tiles per chunk
                NCHUNKS = NH // NCH

                for ch in range(NCHUNKS):
                    last = ch == NCHUNKS - 1
                    pts = []
                    psos = {}
                    inter_nts = (0, 1) if last else (0, 1, 2, 3)
                    for nt in inter_nts:
                        psos[nt] = o_psum.tile([P, C + 1], F32, tag="pso", name=f"pso{nt}")

                    def pv(j, nts):
                        for nt in nts:
                            nc.tensor.matmul(
                                psos[nt],
                                lhsT=pts[j][:, :, ts(nt, P)],
                                rhs=vt_tiles[j],
                                start=(j == 0),
                                stop=(j == PMT - 1),
                                perf_mode=mybir.MatmulPerfMode.DoubleRow,
                            )

                    def finish_nt(nt):
                        rec = r_pool.tile([P, 1], F32, tag="rec", name=f"rec{nt}")
                        nc.vector.reciprocal(out=rec, in_=psos[nt][:, C : C + 1])
                        osb = o_pool.tile([P, C], BF, tag="osb", name=f"osb{nt}")
                        nc.vector.tensor_scalar_mul(out=osb, in0=psos[nt][:, 0:C], scalar1=rec)
                        for cc in range(CT):
                            pst = tf_psum.tile([P, P], BF, tag="psf", name=f"pst{nt}{cc}")
                            nc.tensor.transpose(pst, osb[:, ts(cc, P)], ident_sb)
                            nc.vector.tensor_copy(
                                out=oT_sb[cc][:, ds(ch * NCH + nt * P, P)], in_=pst
                            )

                    for j in range(PMT):
                        pt = p_pool.tile([P, 2, NCH], F8, tag="pt", name=f"pt{j}")
                        for parity in range(2):
                            pss = s_psum.tile([P, NCH], F32, tag="pss")
                            for ct in range(CT):
                                ksl = k_sb[ct][:, ds(j * 2 * P, 2 * P)].rearrange(
                                    "p (m two) -> p two m", two=2
                                )
                                nc.tensor.matmul(
                                    pss,
                                    lhsT=ksl[:, parity, :],
                                    rhs=q_sb[ct][:, ts(ch, NCH)],
                                    start=(ct == 0),
                                    stop=(ct == CT - 1),
                                )
                            nc.scalar.activation(
                                out=pt[:, parity, :], in_=pss, func=AF.Exp, scale=SCALE, bias=nexp_sb
                            )
                        pts.append(pt)
                        if j >= LAG:
                            pv(j - LAG, inter_nts)
                    for j in range(PMT - LAG, PMT):
                        pv(j, inter_nts)
                    if last:
                        psos[2] = o_psum.tile([P, C + 1], F32, tag="pso", name="pso2")
                        for j in range(PMT):
                            pv(j, (2,))
                        finish_nt(0)
                        finish_nt(1)
                        psos[3] = o_psum.tile([P, C + 1], F32, tag="pso", name="pso3")
                        for j in range(PMT):
                            pv(j, (3,))
                        finish_nt(2)
                        finish_nt(3)
                    else:
                        for nt in range(4):
                            finish_nt(nt)
                    # output projection + residual for this chunk
                    for mo in range(CT):
                        psf = tf_psum.tile([P, NCH], F32, tag="psf", name=f"psj{mo}")
                        for ct in range(CT):
                            nc.tensor.matmul(
                                psf,
                                lhsT=wo_sb[:, ct, ts(mo, P)],
                                rhs=oT_sb[ct][:, ts(ch, NCH)],
                                start=(ct == 0),
                                stop=(ct == CT - 1),
                            )
                        fs = out_pool.tile([P, NCH], F32, tag="fs", name=f"fs{mo}")
                        nc.vector.tensor_scalar_add(
                            out=fs, in0=psf, scalar1=bo_sb[:, mo : mo + 1]
                        )
                        nc.vector.tensor_add(out=fs, in0=fs, in1=x_sb[mo][:, ts(ch, NCH)])
                        nc.sync.dma_start(out=out[ts(mo, P), ts(ch, NCH)], in_=fs)

    nc.compile()
    return nc


def get_program():
    if "nc" not in _CACHE:
        _CACHE["nc"] = _build_program()
    return _CACHE["nc"]


def _cpack(bq, bk, bo, gam, bet, bv):
    cp = np.zeros((P, 10 + 16 + P + C + 1), np.float32)
    for j, v in enumerate([bq, bk, bo, gam, bet]):
        cp[:, 2 * j : 2 * j + 2] = v.reshape(CT, P).T
    mfwd = (
        np.arange(P)[:, None] // GSIZE == np.arange(GROUPS // CT)[None, :]
    ).astype(np.float32) / GSIZE
    mbwd = (
        np.arange(GROUPS // CT)[:, None] == np.arange(P)[None, :] // GSIZE
    ).astype(np.float32)
    cp[:, 10:26] = mfwd
    cp[: GROUPS // CT, 26 : 26 + P] = mbwd
    cp[:, 154 : 154 + C] = np.broadcast_to(bv, (P, C))
    cp[:, 154 + C] = 1.0
    return cp


def _make_in_maps(x, gn_gamma, gn_beta, wq, bq, wk, bk, wv, bv, wo, bo):
    f = lambda a: np.ascontiguousarray(np.asarray(a, dtype=np.float32))
    x = f(x).reshape(B, C, N)
    shared = {
        "wqT": f(wq).T.astype(ml_dtypes.bfloat16),
        "wkT": f(wk).T.astype(ml_dtypes.bfloat16),
        "wvTa": np.concatenate(
            [f(wv).T, np.zeros((C, 1), np.float32)], axis=1
        ).astype(ml_dtypes.bfloat16),
        "woT": f(wo).T.astype(ml_dtypes.bfloat16),
        "cpack": _cpack(f(bq), f(bk), f(bo), f(gn_gamma), f(gn_beta), f(bv)),
        "ident": np.eye(P).astype(ml_dtypes.bfloat16),
    }
    in_maps = []
    for core in range(8):
        b, half = core // 2, core % 2
        xbv = x[b]
        if half == 1:
            xbv = np.concatenate([xbv[:, NH:], xbv[:, :NH]], axis=1)
        in_maps.append(
            {
                "xb": np.ascontiguousarray(xbv[:, :NH]),
                "xhb": xbv[:, NH:].astype(ml_dtypes.bfloat16),
                **shared,
            }
        )
    return in_maps


def kernel(**inputs):
    nc = get_program()
    in_maps = _make_in_maps(**inputs)
    res = run_bass_kernel_spmd(nc, in_maps, list(range(8)))
    out = np.empty((B, C, N), dtype=np.float32)
    for core in range(8):
        b, half = core // 2, core % 2
        out[b, :, half * NH : (half + 1) * NH] = res.results[core]["out"]
    return out.reshape(B, C, W, W)
